# revision 25
# baseline (speedup 1.0000x reference)
"""Trainium2 Bass kernel for the Pearson-correlation GNN block.

Pipeline (per sample b, channel c):
  pcc_c = Xn_c^T Xn_c          (Gram of host-normalized columns -> Pearson)
  E     = sum_c pcc_c @ WeT_c  (EdgeConv, accumulated in PSUM)
  e     = leaky_relu(E + b_edge);  pooled = mean_o(e)
  att   = sigmoid(W2 @ relu(W1 @ pooled + b1) + b2)
  o     = leaky_relu(W_node . (att * e) + b_node)

Host precomputes xn = (x - mean_t) / ||x - mean_t||_t transposed to
[B, T, C, R] fp16 so the device Gram directly yields pcc, and lays every
weight out exactly as SBUF wants it. Data parallel: 4 samples per core,
8 cores, all weights replicated.
"""

import numpy as np

from concourse import bacc, bass, mybir, tile
from concourse.tile_rust import add_dep_helper
from concourse.bass_utils import run_bass_kernel_spmd

B, R, T, C = 32, 256, 512, 16
C1, C2 = 64, 128
NCORES = 8
BS = B // NCORES  # samples per core

F16 = mybir.dt.float16
F32 = mybir.dt.float32
AF = mybir.ActivationFunctionType
ALU = mybir.AluOpType

KCH = T // 128   # 4 t-chunks
MCH = R // 128   # 2 r-chunks

_cached_nc = None
_last_results = None


def _build_nc():
    global _cached_nc
    if _cached_nc is not None:
        return _cached_nc

    nc = bacc.Bacc(None)

    xT_d = nc.declare_dram_parameter("xT", [BS, T, C, R], F16, isOutput=False)
    WeT_d = nc.declare_dram_parameter("WeT", [128, 2 * C, C1], F16, isOutput=False)
    WnT_d = nc.declare_dram_parameter("WnT", [128, 2 * C1, C2], F16, isOutput=False)
    W1T_d = nc.declare_dram_parameter("W1T", [128, MCH, C1], F32, isOutput=False)
    W2T_d = nc.declare_dram_parameter("W2T", [C1, MCH, 128], F32, isOutput=False)
    beR_d = nc.declare_dram_parameter("beR", [128, C1], F32, isOutput=False)
    b1_d = nc.declare_dram_parameter("b1c", [C1, 1], F32, isOutput=False)
    b2_d = nc.declare_dram_parameter("b2c", [128, MCH], F32, isOutput=False)
    bn_d = nc.declare_dram_parameter("bnc", [BS, C2], F32, isOutput=False)
    o_d = nc.declare_dram_parameter("o_out", [BS, C2], F32, isOutput=True)
    att_d = nc.declare_dram_parameter("att_out", [BS, R], F32, isOutput=True)

    with tile.TileContext(nc) as tc:
        with (
            tc.tile_pool(name="consts", bufs=1) as consts,
            tc.tile_pool(name="xt", bufs=4 * KCH) as xt_pool,
            tc.tile_pool(name="pcc", bufs=3) as pcc_pool,
            tc.tile_pool(name="work", bufs=2) as work,
            tc.tile_pool(name="psum_pcc", bufs=2, space="PSUM") as psum_pcc,
            tc.tile_pool(name="psum_e", bufs=2, space="PSUM") as psum_e,
            tc.tile_pool(name="psum_mlp", bufs=1, space="PSUM") as psum_mlp,
            tc.tile_pool(name="psum_o", bufs=1, space="PSUM") as psum_o,
        ):
            # Two DMA rings (SP HWDGE + GpSimd SWDGE), each FIFO in program
            # order. Every sample's 4 k-tiles are split 2+2 across the rings
            # so rings stay bandwidth-balanced while FIFO order guarantees
            # sample b lands before sample b+1 with no semaphore games.
            def xt_engine(k):
                return nc.sync if k < KCH // 2 else nc.gpsimd

            # Sample-0 loads come first, in channel halves, so the first
            # Gram matmuls start as soon as ~2MB has landed.
            all_xts = {}
            all_xts[0] = [
                xt_pool.tile([128, C, R], F16, tag="xt", name=f"xt0_{k}")
                for k in range(KCH)
            ]
            for k in range(KCH):
                xt_engine(k).dma_start(
                    out=all_xts[0][k][:, 0 : C // 2, :],
                    in_=xT_d[0, k * 128 : (k + 1) * 128, 0 : C // 2],
                )
            WeT = consts.tile([128, 2 * C, C1], F16)
            nc.sync.dma_start(out=WeT[:], in_=WeT_d[:])
            beR = consts.tile([128, C1], F32)
            nc.gpsimd.dma_start(out=beR[:], in_=beR_d[:])
            for k in range(KCH):
                xt_engine(k).dma_start(
                    out=all_xts[0][k][:, C // 2 : C, :],
                    in_=xT_d[0, k * 128 : (k + 1) * 128, C // 2 : C],
                )
            W1T = consts.tile([128, MCH, C1], F32)
            W2T = consts.tile([C1, MCH, 128], F32)
            b1c = consts.tile([C1, 1], F32)
            b2c = consts.tile([128, MCH], F32)
            bnc = consts.tile([BS, C2], F32)
            WnT = consts.tile([128, 2 * C1, C2], F16)
            late_loads_done = [False]

            def _late_loads():
                if late_loads_done[0]:
                    return
                late_loads_done[0] = True
                nc.sync.dma_start(out=W1T[:], in_=W1T_d[:])
                nc.sync.dma_start(out=W2T[:], in_=W2T_d[:])
                nc.sync.dma_start(out=b1c[:], in_=b1_d[:])
                nc.sync.dma_start(out=b2c[:], in_=b2_d[:])
                nc.sync.dma_start(out=bnc[:], in_=bn_d[:])



            # att-scaled EdgeConv output for all samples: [r-part, rc, o, b]
            A_all = consts.tile([128, MCH, C1, BS], F16)

            gram_anchor = {}

            def emit_edge(c, pcc_sb, e_ps):
                for ih in range(MCH):
                    for m in range(MCH):
                        nc.tensor.matmul(
                            e_ps[m][:],
                            lhsT=pcc_sb[:, ih, m * 128 : (m + 1) * 128],
                            rhs=WeT[:, 2 * c + ih, :],
                            start=(c == 0 and ih == 0),
                            stop=(c == C - 1 and ih == MCH - 1),
                        )

            # HAM warmup: dummy matmuls while the first sample streams in,
            # so the real Grams start at 2.4 GHz. Results are garbage and the
            # first NodeConv matmul (start=True) resets the bank.
            warm_sb = consts.tile([128, C1], F16)
            nc.vector.memset(warm_sb[:], 0.0)
            o_ps = psum_o.tile([BS, C2], F32, tag="o", name="o_ps")
            for _ in range(100):
                nc.tensor.matmul(
                    o_ps[0:BS, 0:C1],
                    lhsT=warm_sb[:, 0:BS],
                    rhs=warm_sb[:],
                    start=True,
                    stop=True,
                )

            mlp_pending = [None]

            def _make_mlp(b, e_ps):
                def emit():
                    # bias + leaky relu + pooled (sum over o via accum_out)
                    pooled = work.tile([128, MCH], F32, tag="pooled", name="pooled")
                    e_sb = work.tile([128, MCH, C1], F32, tag="esb", name="e_sb")
                    for m in range(MCH):
                        t0 = work.tile([128, C1], F32, tag="t0", name="t0")
                        nc.vector.tensor_add(t0[:], e_ps[m][:], beR[:])
                        nc.vector.scalar_tensor_tensor(
                            out=e_sb[:, m, :],
                            in0=t0[:],
                            scalar=0.01,
                            in1=t0[:],
                            op0=ALU.mult,
                            op1=ALU.max,
                            accum_out=pooled[:, m : m + 1],
                        )
                    # MLP (1/64 mean folded into W1T); h and att share one
                    # PSUM bank: h = [0:64, 0:1], att = [:, 1:3]
                    mlp_ps = psum_mlp.tile([128, 3], F32, tag="mlp", name="mlp_ps")
                    for rc in range(MCH):
                        nc.tensor.matmul(
                            mlp_ps[0:C1, 0:1],
                            lhsT=W1T[:, rc, :],
                            rhs=pooled[:, rc : rc + 1],
                            start=(rc == 0),
                            stop=(rc == MCH - 1),
                        )
                    h_sb = work.tile([C1, 1], F32, tag="hsb", name="h_sb")
                    nc.scalar.activation(h_sb[:], mlp_ps[0:C1, 0:1], AF.Relu, bias=b1c[:])
                    for m in range(MCH):
                        nc.tensor.matmul(
                            mlp_ps[:, 1 + m : 2 + m],
                            lhsT=W2T[:, m, :],
                            rhs=h_sb[:],
                            start=True,
                            stop=True,
                        )
                    att_sb = work.tile([128, MCH], F32, tag="attsb", name="att_sb")
                    for m in range(MCH):
                        nc.scalar.activation(
                            att_sb[:, m : m + 1],
                            mlp_ps[:, 1 + m : 2 + m],
                            AF.Sigmoid,
                            bias=b2c[:, m : m + 1],
                        )
                        nc.scalar.dma_start(
                            out=att_d[b, m * 128 : (m + 1) * 128],
                            in_=att_sb[:, m : m + 1],
                        )
                    # A = att * e (fp16, [r-part, rc, o, b] for NodeConv)
                    for m in range(MCH):
                        nc.vector.tensor_scalar_mul(
                            A_all[:, m, :, b], e_sb[:, m, :], att_sb[:, m : m + 1]
                        )
                return emit

            for b in range(BS):
                if b not in all_xts:
                    all_xts[b] = [
                        xt_pool.tile([128, C, R], F16, tag="xt", name=f"xt{b}_{k}")
                        for k in range(KCH)
                    ]
                    for k in range(KCH):
                        xt_engine(k).dma_start(
                            out=all_xts[b][k][:],
                            in_=xT_d[b, k * 128 : (k + 1) * 128],
                        )
                    if b == BS - 1:
                        nc.sync.dma_start(out=WnT[:], in_=WnT_d[:])
                xts = all_xts[b]

                e_ps = [
                    psum_e.tile([128, C1], F32, tag=f"e{m}", name=f"e_ps{m}")
                    for m in range(MCH)
                ]

                # Software-pipelined channel loop: EdgeConv for channel c is
                # emitted after the Gram of c+1 so the in-order PE never
                # stalls on the PSUM->SBUF copy of pcc. The previous sample's
                # MLP tail is emitted two channels in, for the same reason.
                prev = None
                for c in range(C):
                    pcc_ps = psum_pcc.tile([128, MCH, R], F32, tag="pcc")
                    for m in range(MCH):
                        for k in range(KCH):
                            mm = nc.tensor.matmul(
                                pcc_ps[:, m, :],
                                lhsT=xts[k][:, c, m * 128 : (m + 1) * 128],
                                rhs=xts[k][:, c, :],
                                start=(k == 0),
                                stop=(k == KCH - 1),
                            )
                            if m == 0 and k == 0:
                                gram_anchor[(b, c)] = mm.ins
                    pcc_sb = pcc_pool.tile([128, MCH, R], F16, tag="pccsb")
                    if c % 2 == 0:
                        nc.scalar.copy(pcc_sb[:, :, :], pcc_ps[:, :, :])
                    else:
                        nc.vector.tensor_copy(pcc_sb[:, :, :], pcc_ps[:, :, :])
                    if prev is not None:
                        emit_edge(prev[0], prev[1], e_ps)
                    prev = (c, pcc_sb)
                    if c == 2 and mlp_pending[0] is not None:
                        mlp_pending[0]()
                        mlp_pending[0] = None
                emit_edge(prev[0], prev[1], e_ps)

                _late_loads()
                mlp_pending[0] = _make_mlp(b, e_ps)
            mlp_pending[0]()

            # NodeConv: o[b, q] = sum_{o,rc} A_all[:, rc, o, :]^T WnT[:, o*2+rc, :]
            # (stationary = tiny A chunk, so LDWEIGHTS is 4 cols, not 128)
            for o in range(C1):
                for rc in range(MCH):
                    nc.tensor.matmul(
                        o_ps[:],
                        lhsT=A_all[:, rc, o, :],
                        rhs=WnT[:, o * MCH + rc, :],
                        start=(o == 0 and rc == 0),
                        stop=(o == C1 - 1 and rc == MCH - 1),
                    )
            o_t0 = work.tile([BS, C2], F32, tag="ot0")
            nc.vector.tensor_add(o_t0[:], o_ps[:], bnc[:])
            o_sb = work.tile([BS, C2], F32, tag="osb")
            nc.vector.scalar_tensor_tensor(
                out=o_sb[:],
                in0=o_t0[:],
                scalar=0.01,
                in1=o_t0[:],
                op0=ALU.mult,
                op1=ALU.max,
            )
            o_sb2 = work.tile([BS, C2], F32, tag="osb2")
            nc.scalar.copy(o_sb2[:], o_sb[:])
            nc.scalar.dma_start(out=o_d[:], in_=o_sb2[:])

    nc.finalize()
    _cached_nc = nc
    return nc


def prepare_inputs(x, W_edge, b_edge, W1, b1, W2, b2, W_node, b_node):
    """Host-side layout prep. Returns the dict of device input arrays."""
    x = np.asarray(x, dtype=np.float32)
    m = x.mean(axis=2, keepdims=True)
    xc = x - m
    ss = np.sqrt((xc * xc).sum(axis=2, keepdims=True))
    np.divide(xc, ss, out=xc)
    # [B, R, T, C] -> [B, T, C, R] fp16
    xT = np.ascontiguousarray(xc.transpose(0, 2, 3, 1)).astype(np.float16)

    W_edge = np.asarray(W_edge, dtype=np.float32)
    W_node = np.asarray(W_node, dtype=np.float32)
    WeT = (
        W_edge.transpose(1, 2, 0)  # [C, R, C1]
        .reshape(C, MCH, 128, C1)
        .transpose(2, 0, 1, 3)
        .reshape(128, 2 * C, C1)
        .astype(np.float16)
    )
    WnT = (
        W_node.transpose(1, 2, 0)  # [C1, R, C2]
        .reshape(C1, MCH, 128, C2)
        .transpose(2, 0, 1, 3)
        .reshape(128, 2 * C1, C2)
        .astype(np.float16)
    )
    W1T = (
        (np.asarray(W1, dtype=np.float32).T / C1)
        .reshape(MCH, 128, C1)
        .transpose(1, 0, 2)
        .copy()
    )
    W2T = np.asarray(W2, dtype=np.float32).T.reshape(C1, MCH, 128).copy()
    beR = np.broadcast_to(np.asarray(b_edge, dtype=np.float32), (128, C1)).copy()
    b1c = np.asarray(b1, dtype=np.float32).reshape(C1, 1).copy()
    b2c = np.asarray(b2, dtype=np.float32).reshape(MCH, 128).T.copy()
    bnc = np.broadcast_to(np.asarray(b_node, dtype=np.float32), (BS, C2)).copy()
    return {
        "xT": xT,
        "WeT": np.ascontiguousarray(WeT),
        "WnT": np.ascontiguousarray(WnT),
        "W1T": np.ascontiguousarray(W1T),
        "W2T": W2T,
        "beR": beR,
        "b1c": b1c,
        "b2c": b2c,
        "bnc": bnc,
    }


def kernel(x, W_edge, b_edge, W1, b1, W2, b2, W_node, b_node):
    global _last_results
    dev_in = prepare_inputs(x, W_edge, b_edge, W1, b1, W2, b2, W_node, b_node)
    nc = _build_nc()

    shared = {k: v for k, v in dev_in.items() if k != "xT"}
    in_maps = [
        {"xT": np.ascontiguousarray(dev_in["xT"][i * BS : (i + 1) * BS]), **shared}
        for i in range(NCORES)
    ]
    res = run_bass_kernel_spmd(nc, in_maps, list(range(NCORES)))
    _last_results = res

    o = np.concatenate([res.results[i]["o_out"] for i in range(NCORES)], axis=0)
    att = np.concatenate([res.results[i]["att_out"] for i in range(NCORES)], axis=0)
    return (
        o.reshape(B, 1, 1, C2).astype(np.float32),
        att.reshape(B, R, 1, 1).astype(np.float32),
    )


# revision 27
# speedup vs baseline: 1.2530x; 1.2530x over previous
"""Trainium2 Bass kernel for the Pearson-correlation GNN block.

Pipeline (per sample b, channel c):
  pcc_c = Xn_c^T Xn_c          (Gram of host-normalized columns -> Pearson)
  E     = sum_c pcc_c @ WeT_c  (EdgeConv, accumulated in PSUM)
  e     = leaky_relu(E + b_edge);  pooled = mean_o(e)
  att   = sigmoid(W2 @ relu(W1 @ pooled + b1) + b2)
  o     = leaky_relu(W_node . (att * e) + b_node)

Host precomputes xn = (x - mean_t) / ||x - mean_t||_t transposed to
[B, T, C, R] fp16 so the device Gram directly yields pcc, and lays every
weight out exactly as SBUF wants it. Data parallel: 4 samples per core,
8 cores, all weights replicated.
"""

import numpy as np

from concourse import bacc, bass, mybir, tile
from concourse.tile_rust import add_dep_helper
from concourse.bass_utils import run_bass_kernel_spmd

B, R, T, C = 32, 256, 512, 16
C1, C2 = 64, 128
NCORES = 8
BS = B // NCORES  # samples per core

F16 = mybir.dt.float16
F32 = mybir.dt.float32
AF = mybir.ActivationFunctionType
ALU = mybir.AluOpType

KCH = T // 128   # 4 t-chunks
MCH = R // 128   # 2 r-chunks

_cached_nc = None
_last_results = None


def _build_nc():
    global _cached_nc
    if _cached_nc is not None:
        return _cached_nc

    nc = bacc.Bacc(None)

    xT_d = nc.declare_dram_parameter("xT", [BS, T, C, R], F16, isOutput=False)
    WeT_d = nc.declare_dram_parameter("WeT", [128, 2 * C, C1], F16, isOutput=False)
    WnT_d = nc.declare_dram_parameter("WnT", [128, 2 * C1, C2], F16, isOutput=False)
    W1T_d = nc.declare_dram_parameter("W1T", [128, MCH, C1], F32, isOutput=False)
    W2T_d = nc.declare_dram_parameter("W2T", [C1, MCH, 128], F32, isOutput=False)
    beR_d = nc.declare_dram_parameter("beR", [128, C1], F32, isOutput=False)
    b1_d = nc.declare_dram_parameter("b1c", [C1, 1], F32, isOutput=False)
    b2_d = nc.declare_dram_parameter("b2c", [128, MCH], F32, isOutput=False)
    bn_d = nc.declare_dram_parameter("bnc", [BS, C2], F32, isOutput=False)
    o_d = nc.declare_dram_parameter("o_out", [BS, C2], F32, isOutput=True)
    att_d = nc.declare_dram_parameter("att_out", [BS, R], F32, isOutput=True)

    with tile.TileContext(nc) as tc:
        with (
            tc.tile_pool(name="consts", bufs=1) as consts,
            tc.tile_pool(name="xt", bufs=4 * KCH) as xt_pool,
            tc.tile_pool(name="pcc", bufs=3) as pcc_pool,
            tc.tile_pool(name="work", bufs=2) as work,
            tc.tile_pool(name="psum_pcc", bufs=2, space="PSUM") as psum_pcc,
            tc.tile_pool(name="psum_e", bufs=2, space="PSUM") as psum_e,
            tc.tile_pool(name="psum_mlp", bufs=1, space="PSUM") as psum_mlp,
            tc.tile_pool(name="psum_o", bufs=1, space="PSUM") as psum_o,
        ):
            # Two HWDGE rings (SP + ACT), each FIFO in program order. Every
            # sample's 4 k-tiles are split 2+2 across the rings; triggers for
            # sample b+1 are emitted mid-way through sample b's channel loop
            # so their lane-waits never stall the ACT copy stream.
            def xt_engine(k):
                return nc.sync if k < KCH // 2 else nc.scalar

            all_xts = {
                b: [
                    xt_pool.tile([128, C, R], F16, tag="xt", name=f"xt{b}_{k}")
                    for k in range(KCH)
                ]
                for b in range(BS)
            }
            for k in range(KCH):
                xt_engine(k).dma_start(
                    out=all_xts[0][k][:, 0 : C // 2, :],
                    in_=xT_d[0, k * 128 : (k + 1) * 128, 0 : C // 2],
                )
            WeT = consts.tile([128, 2 * C, C1], F16)
            nc.sync.dma_start(out=WeT[:], in_=WeT_d[:])
            beR = consts.tile([128, C1], F32)
            nc.gpsimd.dma_start(out=beR[:], in_=beR_d[:])
            for k in range(KCH):
                xt_engine(k).dma_start(
                    out=all_xts[0][k][:, C // 2 : C, :],
                    in_=xT_d[0, k * 128 : (k + 1) * 128, C // 2 : C],
                )
            W1T = consts.tile([128, MCH, C1], F32)
            W2T = consts.tile([C1, MCH, 128], F32)
            b1c = consts.tile([C1, 1], F32)
            b2c = consts.tile([128, MCH], F32)
            bnc = consts.tile([BS, C2], F32)
            WnT = consts.tile([128, 2 * C1, C2], F16)
            late_loads_done = [False]

            def _late_loads():
                if late_loads_done[0]:
                    return
                late_loads_done[0] = True
                nc.sync.dma_start(out=W1T[:], in_=W1T_d[:])
                nc.sync.dma_start(out=W2T[:], in_=W2T_d[:])
                nc.sync.dma_start(out=b1c[:], in_=b1_d[:])
                nc.sync.dma_start(out=b2c[:], in_=b2_d[:])
                nc.sync.dma_start(out=bnc[:], in_=bn_d[:])



            # att-scaled EdgeConv output for all samples: [r-part, rc, o, b]
            A_all = consts.tile([128, MCH, C1, BS], F16)

            gram_anchor = {}

            def emit_edge(c, pcc_sb, e_ps):
                for ih in range(MCH):
                    for m in range(MCH):
                        nc.tensor.matmul(
                            e_ps[m][:],
                            lhsT=pcc_sb[:, ih, m * 128 : (m + 1) * 128],
                            rhs=WeT[:, 2 * c + ih, :],
                            start=(c == 0 and ih == 0),
                            stop=(c == C - 1 and ih == MCH - 1),
                        )

            # HAM warmup: dummy matmuls while the first sample streams in,
            # so the real Grams start at 2.4 GHz. Results are garbage and the
            # first NodeConv matmul (start=True) resets the bank.
            warm_sb = consts.tile([128, C1], F16)
            nc.vector.memset(warm_sb[:], 0.0)
            o_ps = psum_o.tile([BS, C2], F32, tag="o", name="o_ps")
            for _ in range(100):
                nc.tensor.matmul(
                    o_ps[0:BS, 0:C1],
                    lhsT=warm_sb[:, 0:BS],
                    rhs=warm_sb[:],
                    start=True,
                    stop=True,
                )

            mlp_pending = [None]

            def _make_mlp(b, e_ps):
                def emit():
                    # bias + leaky relu + pooled (sum over o via accum_out)
                    pooled = work.tile([128, MCH], F32, tag="pooled", name="pooled")
                    e_sb = work.tile([128, MCH, C1], F32, tag="esb", name="e_sb")
                    for m in range(MCH):
                        t0 = work.tile([128, C1], F32, tag="t0", name="t0")
                        nc.vector.tensor_add(t0[:], e_ps[m][:], beR[:])
                        nc.vector.scalar_tensor_tensor(
                            out=e_sb[:, m, :],
                            in0=t0[:],
                            scalar=0.01,
                            in1=t0[:],
                            op0=ALU.mult,
                            op1=ALU.max,
                            accum_out=pooled[:, m : m + 1],
                        )
                    # MLP (1/64 mean folded into W1T); h and att share one
                    # PSUM bank: h = [0:64, 0:1], att = [:, 1:3]
                    mlp_ps = psum_mlp.tile([128, 3], F32, tag="mlp", name="mlp_ps")
                    for rc in range(MCH):
                        nc.tensor.matmul(
                            mlp_ps[0:C1, 0:1],
                            lhsT=W1T[:, rc, :],
                            rhs=pooled[:, rc : rc + 1],
                            start=(rc == 0),
                            stop=(rc == MCH - 1),
                        )
                    h_sb = work.tile([C1, 1], F32, tag="hsb", name="h_sb")
                    nc.scalar.activation(h_sb[:], mlp_ps[0:C1, 0:1], AF.Relu, bias=b1c[:])
                    for m in range(MCH):
                        nc.tensor.matmul(
                            mlp_ps[:, 1 + m : 2 + m],
                            lhsT=W2T[:, m, :],
                            rhs=h_sb[:],
                            start=True,
                            stop=True,
                        )
                    att_sb = work.tile([128, MCH], F32, tag="attsb", name="att_sb")
                    for m in range(MCH):
                        nc.scalar.activation(
                            att_sb[:, m : m + 1],
                            mlp_ps[:, 1 + m : 2 + m],
                            AF.Sigmoid,
                            bias=b2c[:, m : m + 1],
                        )
                        nc.scalar.dma_start(
                            out=att_d[b, m * 128 : (m + 1) * 128],
                            in_=att_sb[:, m : m + 1],
                        )
                    # A = att * e (fp16, [r-part, rc, o, b] for NodeConv)
                    for m in range(MCH):
                        nc.vector.tensor_scalar_mul(
                            A_all[:, m, :, b], e_sb[:, m, :], att_sb[:, m : m + 1]
                        )
                return emit

            def emit_next_loads(b):
                nb = b + 1
                if nb < BS:
                    for k in range(KCH):
                        xt_engine(k).dma_start(
                            out=all_xts[nb][k][:],
                            in_=xT_d[nb, k * 128 : (k + 1) * 128],
                        )
                if nb == BS - 1:
                    nc.sync.dma_start(out=WnT[:], in_=WnT_d[:])

            for b in range(BS):
                xts = all_xts[b]

                e_ps = [
                    psum_e.tile([128, C1], F32, tag=f"e{m}", name=f"e_ps{m}")
                    for m in range(MCH)
                ]

                # Software-pipelined channel loop: EdgeConv for channel c is
                # emitted after the Gram of c+1 so the in-order PE never
                # stalls on the PSUM->SBUF copy of pcc. The previous sample's
                # MLP tail is emitted two channels in, for the same reason.
                prev = None
                for c in range(C):
                    pcc_ps = psum_pcc.tile([128, MCH, R], F32, tag="pcc")
                    for m in range(MCH):
                        for k in range(KCH):
                            mm = nc.tensor.matmul(
                                pcc_ps[:, m, :],
                                lhsT=xts[k][:, c, m * 128 : (m + 1) * 128],
                                rhs=xts[k][:, c, :],
                                start=(k == 0),
                                stop=(k == KCH - 1),
                            )
                            if m == 0 and k == 0:
                                gram_anchor[(b, c)] = mm.ins
                    pcc_sb = pcc_pool.tile([128, MCH, R], F16, tag="pccsb")
                    if c % 2 == 0:
                        nc.scalar.copy(pcc_sb[:, :, :], pcc_ps[:, :, :])
                    else:
                        nc.vector.tensor_copy(pcc_sb[:, :, :], pcc_ps[:, :, :])
                    if prev is not None:
                        emit_edge(prev[0], prev[1], e_ps)
                    prev = (c, pcc_sb)
                    if c == 2 and mlp_pending[0] is not None:
                        mlp_pending[0]()
                        mlp_pending[0] = None
                    if c == 6:
                        emit_next_loads(b)
                emit_edge(prev[0], prev[1], e_ps)

                _late_loads()
                mlp_pending[0] = _make_mlp(b, e_ps)
            mlp_pending[0]()

            # NodeConv: o[b, q] = sum_{o,rc} A_all[:, rc, o, :]^T WnT[:, o*2+rc, :]
            # (stationary = tiny A chunk, so LDWEIGHTS is 4 cols, not 128)
            for o in range(C1):
                for rc in range(MCH):
                    nc.tensor.matmul(
                        o_ps[:],
                        lhsT=A_all[:, rc, o, :],
                        rhs=WnT[:, o * MCH + rc, :],
                        start=(o == 0 and rc == 0),
                        stop=(o == C1 - 1 and rc == MCH - 1),
                    )
            o_t0 = work.tile([BS, C2], F32, tag="ot0")
            nc.vector.tensor_add(o_t0[:], o_ps[:], bnc[:])
            o_sb = work.tile([BS, C2], F32, tag="osb")
            nc.vector.scalar_tensor_tensor(
                out=o_sb[:],
                in0=o_t0[:],
                scalar=0.01,
                in1=o_t0[:],
                op0=ALU.mult,
                op1=ALU.max,
            )
            o_sb2 = work.tile([BS, C2], F32, tag="osb2")
            nc.scalar.copy(o_sb2[:], o_sb[:])
            nc.scalar.dma_start(out=o_d[:], in_=o_sb2[:])

    nc.finalize()
    _cached_nc = nc
    return nc


def prepare_inputs(x, W_edge, b_edge, W1, b1, W2, b2, W_node, b_node):
    """Host-side layout prep. Returns the dict of device input arrays."""
    x = np.asarray(x, dtype=np.float32)
    m = x.mean(axis=2, keepdims=True)
    xc = x - m
    ss = np.sqrt((xc * xc).sum(axis=2, keepdims=True))
    np.divide(xc, ss, out=xc)
    # [B, R, T, C] -> [B, T, C, R] fp16
    xT = np.ascontiguousarray(xc.transpose(0, 2, 3, 1)).astype(np.float16)

    W_edge = np.asarray(W_edge, dtype=np.float32)
    W_node = np.asarray(W_node, dtype=np.float32)
    WeT = (
        W_edge.transpose(1, 2, 0)  # [C, R, C1]
        .reshape(C, MCH, 128, C1)
        .transpose(2, 0, 1, 3)
        .reshape(128, 2 * C, C1)
        .astype(np.float16)
    )
    WnT = (
        W_node.transpose(1, 2, 0)  # [C1, R, C2]
        .reshape(C1, MCH, 128, C2)
        .transpose(2, 0, 1, 3)
        .reshape(128, 2 * C1, C2)
        .astype(np.float16)
    )
    W1T = (
        (np.asarray(W1, dtype=np.float32).T / C1)
        .reshape(MCH, 128, C1)
        .transpose(1, 0, 2)
        .copy()
    )
    W2T = np.asarray(W2, dtype=np.float32).T.reshape(C1, MCH, 128).copy()
    beR = np.broadcast_to(np.asarray(b_edge, dtype=np.float32), (128, C1)).copy()
    b1c = np.asarray(b1, dtype=np.float32).reshape(C1, 1).copy()
    b2c = np.asarray(b2, dtype=np.float32).reshape(MCH, 128).T.copy()
    bnc = np.broadcast_to(np.asarray(b_node, dtype=np.float32), (BS, C2)).copy()
    return {
        "xT": xT,
        "WeT": np.ascontiguousarray(WeT),
        "WnT": np.ascontiguousarray(WnT),
        "W1T": np.ascontiguousarray(W1T),
        "W2T": W2T,
        "beR": beR,
        "b1c": b1c,
        "b2c": b2c,
        "bnc": bnc,
    }


def kernel(x, W_edge, b_edge, W1, b1, W2, b2, W_node, b_node):
    global _last_results
    dev_in = prepare_inputs(x, W_edge, b_edge, W1, b1, W2, b2, W_node, b_node)
    nc = _build_nc()

    shared = {k: v for k, v in dev_in.items() if k != "xT"}
    in_maps = [
        {"xT": np.ascontiguousarray(dev_in["xT"][i * BS : (i + 1) * BS]), **shared}
        for i in range(NCORES)
    ]
    res = run_bass_kernel_spmd(nc, in_maps, list(range(NCORES)))
    _last_results = res

    o = np.concatenate([res.results[i]["o_out"] for i in range(NCORES)], axis=0)
    att = np.concatenate([res.results[i]["att_out"] for i in range(NCORES)], axis=0)
    return (
        o.reshape(B, 1, 1, C2).astype(np.float32),
        att.reshape(B, R, 1, 1).astype(np.float32),
    )
